# revision 3
# baseline (speedup 1.0000x reference)
"""CGMM layer-0 forward on 8 Trainium2 NeuronCores — v5.

Counts via a threshold ladder: A[g, thr] = #{t: x >= thr} - T and
    -lik[g, :] = sum_{thr=1..32} A[g, thr] * Dneg[thr-1],
Dneg[r] = L[r] - L[r+1], L[32] := 0 (pads = 64 cancel exactly).

v5: blocks 1-4 (graph sizes <= 128) run ENTIRELY on the column path:
  xpT [128 slots, 512 graph-cols]; per threshold ONE stream instruction
  (x >= thr) - 1 on DVE (~193ns, 4x mode) or Pool (~427ns), reduced by
  a PE matmul with a sliding ones-column stationary into SP[thr-1, :].
  That kills the per-block row-path entirely for j >= 1: no CNT cols,
  no consolidation, no transposes, no ct copies; each block's GEMM is
  one matmul SPs-slice x Dneg.
Block 0 (sizes up to 144) stays on the row path: DVE is_ge+accum
  thr 1..30 (scalar2 seeds the accumulator with -T_0), ACT sign+accum
  thr 31..32 with a (x - T_0) * 0.5 consolidation.
ACT table discipline: warm-up Exp at t=0 (load hidden in the DMA wait);
  exps; block-0 signs (sign lives in the exp set -> no switch); ONE
  switch at lnS; copies thereafter.  Reciprocals are DVE-native.
Lxb (the shifted table) rides a PE identity-slice matmul.
All five GEMMs write one PSUM tile of[:, j, :]; block 0's result DMAs
  early, blocks 1-4 leave in one copy + one DMA.
"""

import math

import numpy as np

N_NODES = 500_000
N_GRAPHS = 5_000
C = 16
M = 32
G = 16
N_CORES = 8
GPC = N_GRAPHS // N_CORES
J = 5
GPAD = 128 * J
PAD_LABEL = 64.0

D0_HI = 30                 # block 0 row path: DVE thr 1..30
A_LO, A_HI = 31, 32        # block 0 row path: ACT thr 31..32
DS_HI = 19                 # col path: DVE streams thr 1..19
NCOLS = 512


def _build_nc(T_blocks):
    import concourse.bass as bass
    import concourse.bacc as bacc
    import concourse.tile as tile
    import concourse.mybir as mybir
    from concourse.masks import make_identity

    fp32 = mybir.dt.float32
    bf16 = mybir.dt.bfloat16
    Alu = mybir.AluOpType
    Act = mybir.ActivationFunctionType

    nc = bacc.Bacc("TRN2", target_bir_lowering=False, debug=False)

    TSUM = sum(T_blocks)
    Toff = [sum(T_blocks[:j]) for j in range(J)]
    T0 = T_blocks[0]

    xp_d = nc.dram_tensor("xp", [128, TSUM], bf16, kind="ExternalInput").ap()
    xpt_d = nc.dram_tensor("xpt", [128, NCOLS], bf16, kind="ExternalInput").ap()
    par_d = nc.dram_tensor("par", [C, G * M + G], fp32, kind="ExternalInput").ap()
    out_d = nc.dram_tensor("out", [GPAD, G], fp32, kind="ExternalOutput").ap()
    out_v = out_d.rearrange("(j p) g -> p j g", p=128)

    with tile.TileContext(nc) as tc:
        with (
            tc.tile_pool(name="main", bufs=1) as main,
            tc.tile_pool(name="psum", bufs=1, space="PSUM") as ps,
        ):
            # ---- input DMAs ----
            XP = main.tile([128, TSUM], bf16)
            XPT = main.tile([128, NCOLS], bf16)
            Par = main.tile([C, G * M + G], fp32)
            nc.gpsimd.dma_start(out=Par, in_=par_d)
            nc.sync.dma_start(out=XP, in_=xp_d)
            nc.sync.dma_start(out=XPT, in_=xpt_d)
            Bt = Par[:, 0 : G * M]
            Pit = Par[:, G * M : G * M + G]

            # ---- ACT warm-up Exp (table load hidden in DMA wait) ----
            warm = main.tile([1, 1], fp32)
            nc.gpsimd.memset(warm, 0.0)
            warms = main.tile([1, 1], fp32)
            nc.scalar.activation(warms, warm, Act.Exp)

            # ---- constants ----
            ones = main.tile([C, G], fp32)
            nc.gpsimd.memset(ones, 1.0)
            identF = main.tile([128, 128], fp32)
            make_identity(nc, identF)
            win = main.tile([128, 64], bf16)
            nc.gpsimd.memset(win, 0.0)
            nc.gpsimd.memset(win[:, 32:33], 1.0)
            sgbias = {}
            for thr in range(A_LO, A_HI + 1):
                sgb = main.tile([128, 1], fp32, tag=f"sgb{thr}", name=f"sgb{thr}")
                nc.gpsimd.memset(sgb, 0.5 - thr)
                sgbias[thr] = sgb

            CNT = main.tile([128, 32], fp32)      # block 0 only
            CNTa = main.tile([128, 2], fp32)

            # ---- ACT: exps + block-0 signs (same table set) ----
            expPi = main.tile([C, G], fp32)
            nc.scalar.activation(expPi, Pit, Act.Exp)
            expB = main.tile([C, G * M], fp32)
            nc.scalar.activation(expB, Bt, Act.Exp)

            spi = ps.tile([1, G], fp32, tag="spi")
            nc.tensor.matmul(spi, ones[:, 0:1], expPi, start=True, stop=True)
            rspi = main.tile([1, G], fp32)
            nc.vector.reciprocal(rspi, spi)
            rspiB = ps.tile([C, G], fp32, tag="rspiB")
            nc.tensor.matmul(rspiB, ones[0:1, :], rspi, start=True, stop=True)

            dscr = [main.tile([128, T0], bf16, tag=f"dscr{i}", name=f"dscr{i}") for i in range(4)]
            ascr = [main.tile([128, T0], bf16, tag=f"ascr{i}", name=f"ascr{i}") for i in range(4)]
            dtsc = [main.tile([128, NCOLS], bf16, tag=f"dtsc{i}", name=f"dtsc{i}") for i in range(4)]
            ptsc = [main.tile([128, NCOLS], bf16, tag=f"ptsc{i}", name=f"ptsc{i}") for i in range(4)]

            xpb0 = XP[:, 0:T0]

            def dve_thr0(thr):
                nc.vector.tensor_scalar(
                    out=dscr[thr % 4][:, 0:T0], in0=xpb0,
                    scalar1=float(thr), scalar2=-float(T0),
                    op0=Alu.is_ge, op1=Alu.add,
                    accum_out=CNT[:, thr - 1 : thr],
                )

            def act_thr0(thr):
                nc.scalar.activation(
                    ascr[thr % 4][:, 0:T0], xpb0, Act.Sign,
                    bias=sgbias[thr], scale=1.0,
                    accum_out=CNTa[:, thr - A_LO : thr - A_LO + 1],
                )

            SP = ps.tile([32, NCOLS], fp32, tag="SP")
            n_win = [0]

            def stream(thr, eng_pool):
                st = (ptsc if eng_pool else dtsc)[thr % 4]
                eng = nc.gpsimd if eng_pool else nc.vector
                eng.tensor_scalar(
                    out=st, in0=XPT,
                    scalar1=float(thr), scalar2=-1.0,
                    op0=Alu.is_ge, op1=Alu.add,
                )
                return st

            def window(thr, st):
                i = thr - 1
                nc.tensor.matmul(
                    SP, win[:, 32 - i : 64 - i], st,
                    start=(n_win[0] == 0), stop=(n_win[0] == 31),
                )
                n_win[0] += 1

            # aux tiles
            sumB = main.tile([C, G], fp32)
            eb = main.tile([C, G * M], fp32)
            scaleT = main.tile([C, G], fp32)
            w2 = main.tile([C, G], fp32)
            smPi = main.tile([C, G], fp32)
            rsumB = main.tile([C, G], fp32)
            lnS = main.tile([C, G], fp32)
            sumH1 = main.tile([C, G, M // 2], fp32)
            sumH2 = main.tile([C, G, M // 8], fp32)

            def pool_sumb():
                ev = expB.rearrange("c (g m) -> c g m", m=M)
                nc.gpsimd.tensor_add(sumH1, ev[:, :, 0 : M // 2], ev[:, :, M // 2 : M])
                nc.gpsimd.tensor_add(
                    sumH1[:, :, 0 : M // 4], sumH1[:, :, 0 : M // 4],
                    sumH1[:, :, M // 4 : M // 2],
                )
                nc.gpsimd.tensor_add(
                    sumH2, sumH1[:, :, 0 : M // 8], sumH1[:, :, M // 8 : M // 4]
                )
                nc.gpsimd.tensor_add(
                    sumH2[:, :, 0:2], sumH2[:, :, 0:2], sumH2[:, :, 2:4]
                )
                nc.gpsimd.tensor_add(sumB, sumH2[:, :, 0:1], sumH2[:, :, 1:2])

            # ---- Pool queue: par DMA, consts, eb, sumB tree, then streams
            # thr 20..32, with scaleT/w2/cons slotted between ----
            nc.gpsimd.tensor_mul(eb, expB, Bt)
            pool_sumb()
            p_streams = []
            for k, thr in enumerate(range(DS_HI + 1, 33)):
                st = stream(thr, eng_pool=True)
                p_streams.append((thr, st))
                if k == 1:
                    nc.gpsimd.tensor_mul(scaleT, smPi, rsumB)
                elif k == 3:
                    nc.gpsimd.tensor_mul(w2, scaleT, lnS)
                elif k == 5:
                    # block-0 sign consolidation: (x - T0) * 0.5
                    nc.gpsimd.tensor_scalar(
                        out=CNT[:, A_LO - 1 : A_HI], in0=CNTa,
                        scalar1=-float(T0), scalar2=0.5,
                        op0=Alu.add, op1=Alu.mult,
                    )

            # ---- ACT queue: block-0 signs right after exps ----
            for thr in range(A_LO, A_HI + 1):
                act_thr0(thr)
            nc.scalar.activation(lnS, scaleT, Act.Ln)

            # PSUM tiles for the PE section (allocated early; the PE
            # instructions are emitted further down)
            z32 = ps.tile([M, G], fp32, tag="z32")
            zp32 = ps.tile([M, G], fp32, tag="zp32")
            LxbP = ps.tile([M, G], fp32, tag="z32")

            # ---- DVE queue: block-0 ladder, aux, then streams 1..19 ----
            for thr in range(1, D0_HI + 1):
                dve_thr0(thr)
                if thr == 12:
                    nc.vector.tensor_mul(smPi, expPi, rspiB)
                    nc.vector.reciprocal(rsumB, sumB)
            d_streams = []
            rz32 = main.tile([M, G], fp32)
            Lxa = main.tile([M, G], fp32)
            Lxb = main.tile([M, G], fp32)
            Dneg = main.tile([M, G], fp32)
            for k, thr in enumerate(range(1, DS_HI + 1)):
                st = stream(thr, eng_pool=False)
                d_streams.append((thr, st))
                if k == 6:
                    nc.vector.reciprocal(rz32, z32)
                    nc.vector.tensor_mul(Lxa, zp32, rz32)
                elif k == 10:
                    nc.vector.tensor_scalar_add(Lxb, LxbP, 0.0)
                    nc.vector.tensor_sub(Dneg, Lxa, Lxb)

            # ---- PE queue: interleave windows with the z matmuls, the
            # Lxb shift, block-0 transpose, and the GEMMs ----
            expBv = expB.rearrange("c (g m) -> c g m", m=M)
            ebv = eb.rearrange("c (g m) -> c g m", m=M)

            # first two Pool windows
            window(*p_streams[0])
            window(*p_streams[1])
            # z / zp matmuls (deps: scaleT, eb, w2)
            for g in range(G):
                nc.tensor.matmul(
                    z32[:, g : g + 1], expBv[:, g, :], scaleT[:, g : g + 1],
                    start=True, stop=True,
                )
                nc.tensor.matmul(
                    zp32[:, g : g + 1], ebv[:, g, :], scaleT[:, g : g + 1],
                    start=True, stop=False,
                )
                nc.tensor.matmul(
                    zp32[:, g : g + 1], expBv[:, g, :], w2[:, g : g + 1],
                    start=False, stop=True,
                )
            # a few more Pool windows, then the Lxb shift matmul
            for pw in p_streams[2:6]:
                window(*pw)
            nc.tensor.matmul(LxbP, identF[0:M, 1 : M + 1], Lxa, start=True, stop=True)
            for pw in p_streams[6:]:
                window(*pw)
            # block-0 transpose once its CNT is complete
            tp0 = ps.tile([32, 128], fp32, tag="tp0")
            nc.tensor.transpose(tp0, CNT, identF)
            for dw in d_streams:
                window(*dw)

            # ---- tails ----
            ct0 = main.tile([32, 128], fp32)
            nc.scalar.copy(ct0, tp0)
            of = ps.tile([128, J, G], fp32, tag="of")
            nc.tensor.matmul(of[:, 0, :], ct0, Dneg, start=True, stop=True)
            OUTS = main.tile([128, J, G], fp32)
            nc.vector.tensor_scalar_add(OUTS[:, 0, :], of[:, 0, :], 0.0)
            nc.sync.dma_start(out=out_v[:, 0, :], in_=OUTS[:, 0, :])

            SPs = main.tile([32, NCOLS], fp32)
            nc.vector.tensor_scalar_add(SPs, SP, 0.0)
            for j in range(1, J):
                nc.tensor.matmul(
                    of[:, j, :], SPs[:, (j - 1) * 128 : j * 128], Dneg,
                    start=True, stop=True,
                )
            nc.vector.tensor_scalar_add(OUTS[:, 1:J, :], of[:, 1:J, :], 0.0)
            nc.sync.dma_start(out=out_v[:, 1:J, :], in_=OUTS[:, 1:J, :])

    nc.compile()
    return nc


def _host_pack(x, batch):
    import ml_dtypes

    sizes = np.bincount(batch, minlength=N_GRAPHS)
    T = max(32, int(math.ceil(sizes.max() / 16.0)) * 16)
    xp = np.full((N_GRAPHS, T), PAD_LABEL, dtype=np.float32)
    mask = np.arange(T)[None, :] < sizes[:, None]
    xp[mask] = x.astype(np.float32)

    orders = []
    block_max = np.zeros(J, dtype=np.int64)
    core_rows = []
    for i in range(N_CORES):
        s = sizes[i * GPC : (i + 1) * GPC]
        order = np.argsort(-s, kind="stable")
        orders.append(order)
        xs = xp[i * GPC : (i + 1) * GPC][order]
        pad = np.full((GPAD - GPC, T), PAD_LABEL, dtype=np.float32)
        xs = np.concatenate([xs, pad], axis=0)
        core_rows.append(xs)
        ss = np.concatenate([s[order], np.zeros(GPAD - GPC, dtype=s.dtype)])
        for j in range(J):
            block_max[j] = max(block_max[j], ss[j * 128 : (j + 1) * 128].max())
    T_blocks = [max(16, int(math.ceil(bm / 8.0)) * 8) for bm in block_max]
    assert block_max[1] <= 128, "col layout requires blocks 1-4 sizes <= 128"

    shards = []
    shardsT = []
    for i in range(N_CORES):
        xs = core_rows[i]
        parts = [xs[j * 128 : (j + 1) * 128, 0 : T_blocks[j]] for j in range(J)]
        packed = np.concatenate(parts, axis=1)
        shards.append(np.ascontiguousarray(packed.astype(ml_dtypes.bfloat16)))
        colsrc = xs[128:GPAD, 0:128]
        xpt = np.ascontiguousarray(colsrc.T.astype(ml_dtypes.bfloat16))
        shardsT.append(xpt)
    return shards, shardsT, T_blocks, orders


def _host_par(B, Pi):
    Bt = np.ascontiguousarray(np.transpose(B, (0, 2, 1))).reshape(C, G * M)
    return np.ascontiguousarray(
        np.concatenate([Bt, Pi], axis=1).astype(np.float32)
    )


def kernel(x, edge_index, batch, B, Pi):
    from concourse.bass_utils import run_bass_kernel_spmd

    x = np.asarray(x).astype(np.int64)
    batch = np.asarray(batch).astype(np.int64)
    B = np.asarray(B, dtype=np.float32)
    Pi = np.asarray(Pi, dtype=np.float32)

    shards, shardsT, T_blocks, orders = _host_pack(x, batch)

    nc = _build_nc(T_blocks)

    par = _host_par(B, Pi)
    in_maps = [
        {"xp": shards[i], "xpt": shardsT[i], "par": par} for i in range(N_CORES)
    ]

    res = run_bass_kernel_spmd(
        nc, in_maps, core_ids=list(range(N_CORES)), **_RUN_KWARGS
    )
    kernel.last_results = res
    parts = []
    for i in range(N_CORES):
        o_sorted = res.results[i]["out"][:GPC]
        o = np.empty_like(o_sorted)
        o[orders[i]] = o_sorted
        parts.append(o)
    out = np.concatenate(parts)
    return out[:, None, :].astype(np.float32)


_RUN_KWARGS = {}            # ---- Pool: first streams need only xpT (~2.5us) ----
            p_iter = iter(range(DS_HI + 1, 33))
            def pool_stream_w():
                thr = next(p_iter, None)
                if thr is not None:
                    window(thr, stream(thr, True))

            pool_stream_w()
            pool_stream_w()
            # eb + sumB tree next on Pool (expB lands ~3.2)
            nc.gpsimd.tensor_mul(eb, expB, Bt)
            pool_sumb()

            # ---- DVE: block-0 ladder 1..12 + the scale chain ----
            for thr in range(1, 13):
                dve_thr0(thr)
            nc.vector.tensor_mul(smPi, expPi, rspiB)
            nc.vector.reciprocal(rsumB, sumB)
            nc.vector.tensor_mul(scaleT, smPi, rsumB)

            # ---- PE: z32 (needs scaleT only) ----
            for g in range(G):
                nc.tensor.matmul(
                    z32[:, g : g + 1], expBv[:, g, :], scaleT[:, g : g + 1],
                    start=True, stop=True,
                )

            # ---- ACT: block-0 signs, then the single table switch ----
            for thr in range(A_LO, A_HI + 1):
                act_thr0(thr)
            nc.scalar.activation(lnS, scaleT, Act.Ln)
            nc.vector.tensor_mul(w2, scaleT, lnS)

            # ---- PE: zp32 (needs eb, w2) ----
            for g in range(G):
                nc.tensor.matmul(
                    zp32[:, g : g + 1], ebv[:, g, :], scaleT[:, g : g + 1],
                    start=True, stop=False,
                )
                nc.tensor.matmul(
                    zp32[:, g : g + 1], expBv[:, g, :], w2[:, g : g + 1],
                    start=False, stop=True,
                )

            # ---- interleave: block-0 ladder 13..30 with d-streams and
            # the remaining Pool streams; L chain at the seams ----
            for i in range(18):
                dve_thr0(13 + i)
                window(i + 1, stream(i + 1, False))
                if i % 2 == 0:
                    pool_stream_w()
                if i == 4:
                    nc.vector.reciprocal(rz32, z32)
                elif i == 8:
                    nc.vector.tensor_mul(Lxa, zp32, rz32)
                    nc.tensor.matmul(
                        LxbP, identF[0:M, 1 : M + 1], Lxa,
                        start=True, stop=True,
                    )
                elif i == 11:
                    nc.vector.tensor_scalar_add(Lxb, LxbP, 0.0)
                    nc.vector.tensor_sub(Dneg, Lxa, Lxb)
            window(DS_HI, stream(DS_HI, False))
            while True:
                thr = next(p_iter, None)
                if thr is None:
                    break
                window(thr, stream(thr, True))

            # ---- block-0 tail ----
            nc.gpsimd.tensor_scalar(
                out=CNT[:, A_LO - 1 : A_HI], in0=CNTa,
                scalar1=-float(T0), scalar2=0.5,
                op0=Alu.add, op1=Alu.mult,
            )
            tp0 = ps.tile([32, 128], fp32, tag="tp0")
            nc.tensor.transpose(tp0, CNT, identF)
            ct0 = main.tile([32, 128], fp32)
            nc.scalar.copy(ct0, tp0)
            of = ps.tile([128, J, G], fp32, tag="of")
            nc.tensor.matmul(of[:, 0, :], ct0, Dneg, start=True, stop=True)
            OUTS = main.tile([128, J, G], fp32)
            nc.vector.tensor_scalar_add(OUTS[:, 0, :], of[:, 0, :], 0.0)
            nc.sync.dma_start(out=out_v[:, 0, :], in_=OUTS[:, 0, :])




# revision 4
# speedup vs baseline: 1.0099x; 1.0099x over previous
"""CGMM layer-0 forward on 8 Trainium2 NeuronCores — v5.

Counts via a threshold ladder: A[g, thr] = #{t: x >= thr} - T and
    -lik[g, :] = sum_{thr=1..32} A[g, thr] * Dneg[thr-1],
Dneg[r] = L[r] - L[r+1], L[32] := 0 (pads = 64 cancel exactly).

v5: blocks 1-4 (graph sizes <= 128) run ENTIRELY on the column path:
  xpT [128 slots, 512 graph-cols]; per threshold ONE stream instruction
  (x >= thr) - 1 on DVE (~193ns, 4x mode) or Pool (~427ns), reduced by
  a PE matmul with a sliding ones-column stationary into SP[thr-1, :].
  That kills the per-block row-path entirely for j >= 1: no CNT cols,
  no consolidation, no transposes, no ct copies; each block's GEMM is
  one matmul SPs-slice x Dneg.
Block 0 (sizes up to 144) stays on the row path: DVE is_ge+accum
  thr 1..30 (scalar2 seeds the accumulator with -T_0), ACT sign+accum
  thr 31..32 with a (x - T_0) * 0.5 consolidation.
ACT table discipline: warm-up Exp at t=0 (load hidden in the DMA wait);
  exps; block-0 signs (sign lives in the exp set -> no switch); ONE
  switch at lnS; copies thereafter.  Reciprocals are DVE-native.
Lxb (the shifted table) rides a PE identity-slice matmul.
All five GEMMs write one PSUM tile of[:, j, :]; block 0's result DMAs
  early, blocks 1-4 leave in one copy + one DMA.
"""

import math

import numpy as np

N_NODES = 500_000
N_GRAPHS = 5_000
C = 16
M = 32
G = 16
N_CORES = 8
GPC = N_GRAPHS // N_CORES
J = 5
GPAD = 128 * J
PAD_LABEL = 64.0

D0_HI = 30                 # block 0 row path: DVE thr 1..30
A_LO, A_HI = 31, 32        # block 0 row path: ACT thr 31..32
DS_HI = 19                 # col path: DVE streams thr 1..19
NCOLS = 512


def _build_nc(T_blocks):
    import concourse.bass as bass
    import concourse.bacc as bacc
    import concourse.tile as tile
    import concourse.mybir as mybir
    from concourse.masks import make_identity

    fp32 = mybir.dt.float32
    bf16 = mybir.dt.bfloat16
    Alu = mybir.AluOpType
    Act = mybir.ActivationFunctionType

    nc = bacc.Bacc("TRN2", target_bir_lowering=False, debug=False)

    TSUM = sum(T_blocks)
    Toff = [sum(T_blocks[:j]) for j in range(J)]
    T0 = T_blocks[0]

    xp_d = nc.dram_tensor("xp", [128, TSUM], bf16, kind="ExternalInput").ap()
    xpt_d = nc.dram_tensor("xpt", [128, NCOLS], bf16, kind="ExternalInput").ap()
    par_d = nc.dram_tensor("par", [C, G * M + G], fp32, kind="ExternalInput").ap()
    out_d = nc.dram_tensor("out", [GPAD, G], fp32, kind="ExternalOutput").ap()
    out_v = out_d.rearrange("(j p) g -> p j g", p=128)

    with tile.TileContext(nc) as tc:
        with (
            tc.tile_pool(name="main", bufs=1) as main,
            tc.tile_pool(name="psum", bufs=1, space="PSUM") as ps,
        ):
            # ---- input DMAs ----
            XP = main.tile([128, TSUM], bf16)
            XPT = main.tile([128, NCOLS], bf16)
            Par = main.tile([C, G * M + G], fp32)
            nc.gpsimd.dma_start(out=Par, in_=par_d)
            nc.sync.dma_start(out=XPT, in_=xpt_d)
            nc.sync.dma_start(out=XP, in_=xp_d)
            Bt = Par[:, 0 : G * M]
            Pit = Par[:, G * M : G * M + G]

            # ---- ACT warm-up Exp (table load hidden in DMA wait) ----
            warm = main.tile([1, 1], fp32)
            nc.gpsimd.memset(warm, 0.0)
            warms = main.tile([1, 1], fp32)
            nc.scalar.activation(warms, warm, Act.Exp)

            # ---- constants ----
            ones = main.tile([C, G], fp32)
            nc.gpsimd.memset(ones, 1.0)
            identF = main.tile([128, 128], fp32)
            make_identity(nc, identF)
            win = main.tile([128, 64], bf16)
            nc.gpsimd.memset(win, 0.0)
            nc.gpsimd.memset(win[:, 32:33], 1.0)
            sgbias = {}
            for thr in range(A_LO, A_HI + 1):
                sgb = main.tile([128, 1], fp32, tag=f"sgb{thr}", name=f"sgb{thr}")
                nc.gpsimd.memset(sgb, 0.5 - thr)
                sgbias[thr] = sgb

            CNT = main.tile([128, 32], fp32)      # block 0 only
            CNTa = main.tile([128, 2], fp32)

            # ---- ACT: exps + block-0 signs (same table set) ----
            expPi = main.tile([C, G], fp32)
            nc.scalar.activation(expPi, Pit, Act.Exp)
            expB = main.tile([C, G * M], fp32)
            nc.scalar.activation(expB, Bt, Act.Exp)

            spi = ps.tile([1, G], fp32, tag="spi")
            nc.tensor.matmul(spi, ones[:, 0:1], expPi, start=True, stop=True)
            rspi = main.tile([1, G], fp32)
            nc.vector.reciprocal(rspi, spi)
            rspiB = ps.tile([C, G], fp32, tag="rspiB")
            nc.tensor.matmul(rspiB, ones[0:1, :], rspi, start=True, stop=True)

            dscr = [main.tile([128, T0], bf16, tag=f"dscr{i}", name=f"dscr{i}") for i in range(4)]
            ascr = [main.tile([128, T0], bf16, tag=f"ascr{i}", name=f"ascr{i}") for i in range(4)]
            dtsc = [main.tile([128, NCOLS], bf16, tag=f"dtsc{i}", name=f"dtsc{i}") for i in range(4)]
            ptsc = [main.tile([128, NCOLS], bf16, tag=f"ptsc{i}", name=f"ptsc{i}") for i in range(4)]

            xpb0 = XP[:, 0:T0]

            def dve_thr0(thr):
                nc.vector.tensor_scalar(
                    out=dscr[thr % 4][:, 0:T0], in0=xpb0,
                    scalar1=float(thr), scalar2=-float(T0),
                    op0=Alu.is_ge, op1=Alu.add,
                    accum_out=CNT[:, thr - 1 : thr],
                )

            def act_thr0(thr):
                nc.scalar.activation(
                    ascr[thr % 4][:, 0:T0], xpb0, Act.Sign,
                    bias=sgbias[thr], scale=1.0,
                    accum_out=CNTa[:, thr - A_LO : thr - A_LO + 1],
                )

            SP = ps.tile([32, NCOLS], fp32, tag="SP")
            n_win = [0]

            def stream(thr, eng_pool):
                st = (ptsc if eng_pool else dtsc)[thr % 4]
                eng = nc.gpsimd if eng_pool else nc.vector
                eng.tensor_scalar(
                    out=st, in0=XPT,
                    scalar1=float(thr), scalar2=-1.0,
                    op0=Alu.is_ge, op1=Alu.add,
                )
                return st

            def window(thr, st):
                i = thr - 1
                nc.tensor.matmul(
                    SP, win[:, 32 - i : 64 - i], st,
                    start=(n_win[0] == 0), stop=(n_win[0] == 31),
                )
                n_win[0] += 1

            # aux tiles
            sumB = main.tile([C, G], fp32)
            eb = main.tile([C, G * M], fp32)
            scaleT = main.tile([C, G], fp32)
            w2 = main.tile([C, G], fp32)
            smPi = main.tile([C, G], fp32)
            rsumB = main.tile([C, G], fp32)
            lnS = main.tile([C, G], fp32)
            sumH1 = main.tile([C, G, M // 2], fp32)
            sumH2 = main.tile([C, G, M // 8], fp32)

            def pool_sumb():
                ev = expB.rearrange("c (g m) -> c g m", m=M)
                nc.gpsimd.tensor_add(sumH1, ev[:, :, 0 : M // 2], ev[:, :, M // 2 : M])
                nc.gpsimd.tensor_add(
                    sumH1[:, :, 0 : M // 4], sumH1[:, :, 0 : M // 4],
                    sumH1[:, :, M // 4 : M // 2],
                )
                nc.gpsimd.tensor_add(
                    sumH2, sumH1[:, :, 0 : M // 8], sumH1[:, :, M // 8 : M // 4]
                )
                nc.gpsimd.tensor_add(
                    sumH2[:, :, 0:2], sumH2[:, :, 0:2], sumH2[:, :, 2:4]
                )
                nc.gpsimd.tensor_add(sumB, sumH2[:, :, 0:1], sumH2[:, :, 1:2])

            # ---- Pool queue: par DMA, consts, eb, sumB tree, then streams
            # thr 20..32, with scaleT/w2/cons slotted between ----
            nc.gpsimd.tensor_mul(eb, expB, Bt)
            pool_sumb()
            p_streams = []
            for k, thr in enumerate(range(DS_HI + 1, 33)):
                st = stream(thr, eng_pool=True)
                p_streams.append((thr, st))
                if k == 1:
                    nc.gpsimd.tensor_mul(scaleT, smPi, rsumB)
                elif k == 3:
                    nc.gpsimd.tensor_mul(w2, scaleT, lnS)
                elif k == 5:
                    # block-0 sign consolidation: (x - T0) * 0.5
                    nc.gpsimd.tensor_scalar(
                        out=CNT[:, A_LO - 1 : A_HI], in0=CNTa,
                        scalar1=-float(T0), scalar2=0.5,
                        op0=Alu.add, op1=Alu.mult,
                    )

            # ---- ACT queue: block-0 signs right after exps ----
            for thr in range(A_LO, A_HI + 1):
                act_thr0(thr)
            nc.scalar.activation(lnS, scaleT, Act.Ln)

            # PSUM tiles for the PE section (allocated early; the PE
            # instructions are emitted further down)
            z32 = ps.tile([M, G], fp32, tag="z32")
            zp32 = ps.tile([M, G], fp32, tag="zp32")
            LxbP = ps.tile([M, G], fp32, tag="z32")

            # ---- DVE queue: block-0 ladder, aux, then streams 1..19 ----
            for thr in range(1, D0_HI + 1):
                dve_thr0(thr)
                if thr == 12:
                    nc.vector.tensor_mul(smPi, expPi, rspiB)
                    nc.vector.reciprocal(rsumB, sumB)
            d_streams = []
            rz32 = main.tile([M, G], fp32)
            Lxa = main.tile([M, G], fp32)
            Lxb = main.tile([M, G], fp32)
            Dneg = main.tile([M, G], fp32)
            for k, thr in enumerate(range(1, DS_HI + 1)):
                st = stream(thr, eng_pool=False)
                d_streams.append((thr, st))
                if k == 6:
                    nc.vector.reciprocal(rz32, z32)
                    nc.vector.tensor_mul(Lxa, zp32, rz32)
                elif k == 10:
                    nc.vector.tensor_scalar_add(Lxb, LxbP, 0.0)
                    nc.vector.tensor_sub(Dneg, Lxa, Lxb)

            # ---- PE queue: interleave windows with the z matmuls, the
            # Lxb shift, block-0 transpose, and the GEMMs ----
            expBv = expB.rearrange("c (g m) -> c g m", m=M)
            ebv = eb.rearrange("c (g m) -> c g m", m=M)

            # first two Pool windows
            window(*p_streams[0])
            window(*p_streams[1])
            # z / zp matmuls (deps: scaleT, eb, w2)
            for g in range(G):
                nc.tensor.matmul(
                    z32[:, g : g + 1], expBv[:, g, :], scaleT[:, g : g + 1],
                    start=True, stop=True,
                )
                nc.tensor.matmul(
                    zp32[:, g : g + 1], ebv[:, g, :], scaleT[:, g : g + 1],
                    start=True, stop=False,
                )
                nc.tensor.matmul(
                    zp32[:, g : g + 1], expBv[:, g, :], w2[:, g : g + 1],
                    start=False, stop=True,
                )
            # a few more Pool windows, then the Lxb shift matmul
            for pw in p_streams[2:6]:
                window(*pw)
            nc.tensor.matmul(LxbP, identF[0:M, 1 : M + 1], Lxa, start=True, stop=True)
            for pw in p_streams[6:]:
                window(*pw)
            # block-0 transpose once its CNT is complete
            tp0 = ps.tile([32, 128], fp32, tag="tp0")
            nc.tensor.transpose(tp0, CNT, identF)
            for dw in d_streams:
                window(*dw)

            # ---- tails ----
            ct0 = main.tile([32, 128], fp32)
            nc.scalar.copy(ct0, tp0)
            of = ps.tile([128, J, G], fp32, tag="of")
            nc.tensor.matmul(of[:, 0, :], ct0, Dneg, start=True, stop=True)
            OUTS = main.tile([128, J, G], fp32)
            nc.vector.tensor_scalar_add(OUTS[:, 0, :], of[:, 0, :], 0.0)
            nc.sync.dma_start(out=out_v[:, 0, :], in_=OUTS[:, 0, :])

            SPs = main.tile([32, NCOLS], fp32)
            nc.vector.tensor_scalar_add(SPs, SP, 0.0)
            for j in range(1, J):
                nc.tensor.matmul(
                    of[:, j, :], SPs[:, (j - 1) * 128 : j * 128], Dneg,
                    start=True, stop=True,
                )
            nc.vector.tensor_scalar_add(OUTS[:, 1:J, :], of[:, 1:J, :], 0.0)
            nc.sync.dma_start(out=out_v[:, 1:J, :], in_=OUTS[:, 1:J, :])

    nc.compile()
    return nc


def _host_pack(x, batch):
    import ml_dtypes

    sizes = np.bincount(batch, minlength=N_GRAPHS)
    T = max(32, int(math.ceil(sizes.max() / 16.0)) * 16)
    xp = np.full((N_GRAPHS, T), PAD_LABEL, dtype=np.float32)
    mask = np.arange(T)[None, :] < sizes[:, None]
    xp[mask] = x.astype(np.float32)

    orders = []
    block_max = np.zeros(J, dtype=np.int64)
    core_rows = []
    for i in range(N_CORES):
        s = sizes[i * GPC : (i + 1) * GPC]
        order = np.argsort(-s, kind="stable")
        orders.append(order)
        xs = xp[i * GPC : (i + 1) * GPC][order]
        pad = np.full((GPAD - GPC, T), PAD_LABEL, dtype=np.float32)
        xs = np.concatenate([xs, pad], axis=0)
        core_rows.append(xs)
        ss = np.concatenate([s[order], np.zeros(GPAD - GPC, dtype=s.dtype)])
        for j in range(J):
            block_max[j] = max(block_max[j], ss[j * 128 : (j + 1) * 128].max())
    T_blocks = [max(16, int(math.ceil(bm / 8.0)) * 8) for bm in block_max]
    assert block_max[1] <= 128, "col layout requires blocks 1-4 sizes <= 128"

    shards = []
    shardsT = []
    for i in range(N_CORES):
        xs = core_rows[i]
        parts = [xs[j * 128 : (j + 1) * 128, 0 : T_blocks[j]] for j in range(J)]
        packed = np.concatenate(parts, axis=1)
        shards.append(np.ascontiguousarray(packed.astype(ml_dtypes.bfloat16)))
        colsrc = xs[128:GPAD, 0:128]
        xpt = np.ascontiguousarray(colsrc.T.astype(ml_dtypes.bfloat16))
        shardsT.append(xpt)
    return shards, shardsT, T_blocks, orders


def _host_par(B, Pi):
    Bt = np.ascontiguousarray(np.transpose(B, (0, 2, 1))).reshape(C, G * M)
    return np.ascontiguousarray(
        np.concatenate([Bt, Pi], axis=1).astype(np.float32)
    )


def kernel(x, edge_index, batch, B, Pi):
    from concourse.bass_utils import run_bass_kernel_spmd

    x = np.asarray(x).astype(np.int64)
    batch = np.asarray(batch).astype(np.int64)
    B = np.asarray(B, dtype=np.float32)
    Pi = np.asarray(Pi, dtype=np.float32)

    shards, shardsT, T_blocks, orders = _host_pack(x, batch)

    nc = _build_nc(T_blocks)

    par = _host_par(B, Pi)
    in_maps = [
        {"xp": shards[i], "xpt": shardsT[i], "par": par} for i in range(N_CORES)
    ]

    res = run_bass_kernel_spmd(
        nc, in_maps, core_ids=list(range(N_CORES)), **_RUN_KWARGS
    )
    kernel.last_results = res
    parts = []
    for i in range(N_CORES):
        o_sorted = res.results[i]["out"][:GPC]
        o = np.empty_like(o_sorted)
        o[orders[i]] = o_sorted
        parts.append(o)
    out = np.concatenate(parts)
    return out[:, None, :].astype(np.float32)


_RUN_KWARGS = {}            # ---- Pool: first streams need only xpT (~2.5us) ----
            p_iter = iter(range(DS_HI + 1, 33))
            def pool_stream_w():
                thr = next(p_iter, None)
                if thr is not None:
                    window(thr, stream(thr, True))

            pool_stream_w()
            pool_stream_w()
            # eb + sumB tree next on Pool (expB lands ~3.2)
            nc.gpsimd.tensor_mul(eb, expB, Bt)
            pool_sumb()

            # ---- DVE: block-0 ladder 1..12 + the scale chain ----
            for thr in range(1, 13):
                dve_thr0(thr)
            nc.vector.tensor_mul(smPi, expPi, rspiB)
            nc.vector.reciprocal(rsumB, sumB)
            nc.vector.tensor_mul(scaleT, smPi, rsumB)

            # ---- PE: z32 (needs scaleT only) ----
            for g in range(G):
                nc.tensor.matmul(
                    z32[:, g : g + 1], expBv[:, g, :], scaleT[:, g : g + 1],
                    start=True, stop=True,
                )

            # ---- ACT: block-0 signs, then the single table switch ----
            for thr in range(A_LO, A_HI + 1):
                act_thr0(thr)
            nc.scalar.activation(lnS, scaleT, Act.Ln)
            nc.vector.tensor_mul(w2, scaleT, lnS)

            # ---- PE: zp32 (needs eb, w2) ----
            for g in range(G):
                nc.tensor.matmul(
                    zp32[:, g : g + 1], ebv[:, g, :], scaleT[:, g : g + 1],
                    start=True, stop=False,
                )
                nc.tensor.matmul(
                    zp32[:, g : g + 1], expBv[:, g, :], w2[:, g : g + 1],
                    start=False, stop=True,
                )

            # ---- interleave: block-0 ladder 13..30 with d-streams and
            # the remaining Pool streams; L chain at the seams ----
            for i in range(18):
                dve_thr0(13 + i)
                window(i + 1, stream(i + 1, False))
                if i % 2 == 0:
                    pool_stream_w()
                if i == 4:
                    nc.vector.reciprocal(rz32, z32)
                elif i == 8:
                    nc.vector.tensor_mul(Lxa, zp32, rz32)
                    nc.tensor.matmul(
                        LxbP, identF[0:M, 1 : M + 1], Lxa,
                        start=True, stop=True,
                    )
                elif i == 11:
                    nc.vector.tensor_scalar_add(Lxb, LxbP, 0.0)
                    nc.vector.tensor_sub(Dneg, Lxa, Lxb)
            # block-0 sign consolidation (CNTa ready long ago)
            nc.gpsimd.tensor_scalar(
                out=CNT[:, A_LO - 1 : A_HI], in0=CNTa,
                scalar1=-float(T0), scalar2=0.5,
                op0=Alu.add, op1=Alu.mult,
            )
            window(DS_HI, stream(DS_HI, False))
            while True:
                thr = next(p_iter, None)
                if thr is None:
                    break
                window(thr, stream(thr, True))

            # ---- block-0 tail ----
            tp0 = ps.tile([32, 128], fp32, tag="tp0")
            nc.tensor.transpose(tp0, CNT, identF)
            ct0 = main.tile([32, 128], fp32)
            nc.scalar.copy(ct0, tp0)
            of = ps.tile([128, J, G], fp32, tag="of")
            nc.tensor.matmul(of[:, 0, :], ct0, Dneg, start=True, stop=True)
            OUTS = main.tile([128, J, G], fp32)
            nc.vector.tensor_scalar_add(OUTS[:, 0, :], of[:, 0, :], 0.0)
            nc.sync.dma_start(out=out_v[:, 0, :], in_=OUTS[:, 0, :])


